# revision 15
# baseline (speedup 1.0000x reference)
"""Trainium2 Bass kernel: 3-layer spiking NN (DSNN) forward, 50 timesteps.

Strategy (8 NeuronCores, no inter-core communication inside the time loop):
  - Layer-0 drive H0 = inputs @ W0 is constant over time -> computed once on
    each core in exact fp32 (layer-0 spike trains are chaotically sensitive
    to H0 error), transposed to [feature, batch] layout via PE transposes.
    The PE is warmed up with junk transposes + sacrificial micro-matmuls so
    the H0 fp32 matmuls are costed at the ramped p-state.
  - Per-step work is spread over all four compute engines:
      DVE : layer-0 LIF only (fused custom op, 2 slabs of 1024)
      ACT : spike extraction for features [0:1792) + S1 PSUM->SBUF copy
      Pool: spike extraction [1792:2048) (is_gt), layer-1 LIF as two
            scalar_tensor_tensor primitives, spk1 extraction (is_gt)
      PE  : 16 accumulating fp32r matmuls (spk0 @ W1 shard) plus an
            alpha-scaled identity matmul that folds the synaptic decay
            S1' = alpha*S1 + h1 directly into the PSUM accumulation, and
            the layer-2 A-accumulation matmul (delayed 2 steps in PE
            program order so the cross-engine chain never stalls the PE).
  - Layer 2 is linear: m2(T) = sum_t c_t * (spk1(t) @ W2).  A = sum c_t*spk1
    accumulates in a pinned PSUM bank; one A @ W2[shard, :] matmul at the end.
  - Host sums the 8 partial [128, 512] outputs.
"""

import numpy as np
from contextlib import ExitStack

import concourse.bacc as bacc
import concourse.bass as bass
import concourse.mybir as mybir
import concourse.tile as tile
from concourse import bass_utils
from concourse import dve_ops as _DOPS
from concourse.dve_spec import Spec, Src0, Src1, C0, One, lower as _dve_lower
from concourse.dve_uop import DveOpSpec as _DveOpSpec

ALPHA = 0.9
BETA = 0.85
T = 50
B = 128            # batch
F0, F1, F3 = 1024, 2048, 512
N_CORES = 8
SH = F1 // N_CORES  # 256 layer-1 features per core
KC0 = F0 // 128     # 8 contraction chunks for H0
KC1 = F1 // 128     # 16 contraction chunks for layer-1 matmul
NS = 512            # H0 free-dim slice width
BIG = 1.0e30
SPLIT_B = 1536      # ACT handles spk0 cols [0:SPLIT_B), Pool [SPLIT_B:F1)

f32 = mybir.dt.float32
f32r = mybir.dt.float32r
AL = mybir.AluOpType
AF = mybir.ActivationFunctionType

N_WARM = 16         # PE warmup transposes before the H0 fp32 matmuls


def _register_lif():
    """Fused LIF step on the pre-reset membrane:
    out = beta * (in0 * (in0 <= 1)) + in1  (reset-gate, leak, drive)."""
    name = "DSNN_LIF_STEP"
    for op in _DOPS.OPS:
        if op.name == name:
            return op
    body = (Src0 * (Src0 <= One)) * C0 + Src1
    spec = Spec(body=body,
                reference=lambda in0, in1, s0, s1, imm2:
                    ((in0 * (in0 <= 1.0)) * np.float32(s0) + in1).astype(np.float32))
    row = max(_DOPS._SUB_OPCODE_FOR_NAME.values()) + 1
    _DOPS._SUB_OPCODE_FOR_NAME[name] = row
    shas = {}
    for ver in ("v3", "v4"):
        uops = _dve_lower(spec, ver=ver)
        shas[ver] = _DveOpSpec(name=name, opcode=row, uops=uops, rd1_en=True).sha(ver)
    op = _DOPS.DveOp(name, spec, subdim=False, uops_sha=shas)
    _DOPS.OPS.append(op)
    _DOPS.CUSTOM_DVE_SPECS[name] = spec
    return op


LIF = _register_lif()


def _coeffs():
    # m2(T) = sum_{t=1..T} c[t-1] * h2(t)
    c = np.zeros(T, dtype=np.float64)
    for s in range(T):
        tau = s + 1
        c[s] = sum(BETA ** (T - t) * ALPHA ** (t - tau) for t in range(tau, T + 1))
    return c.astype(np.float32)


def _build():
    nc = bacc.Bacc("TRN2", target_bir_lowering=False, debug=False)
    d_inT = nc.dram_tensor("inT", [F0, B], f32, kind="ExternalInput")
    d_W0 = nc.dram_tensor("W0", [F0, F1], f32, kind="ExternalInput")
    d_W1s = nc.dram_tensor("W1s", [F1, SH], f32r, kind="ExternalInput")
    d_W2s = nc.dram_tensor("W2s", [SH, F3], f32r, kind="ExternalInput")
    d_eye = nc.dram_tensor("EYE", [128, 128], f32, kind="ExternalInput")
    d_out = nc.dram_tensor("OUT", [B, F3], f32, kind="ExternalOutput")

    with tile.TileContext(nc) as tc, ExitStack() as ctx:
        const_pool = ctx.enter_context(tc.tile_pool(name="const", bufs=1))
        state_pool = ctx.enter_context(tc.tile_pool(name="state", bufs=1))
        w0_pool = ctx.enter_context(tc.tile_pool(name="w0s", bufs=3))
        htmp_pool = ctx.enter_context(tc.tile_pool(name="htmp", bufs=2))
        out_pool = ctx.enter_context(tc.tile_pool(name="outp", bufs=1))
        psH_pool = ctx.enter_context(tc.tile_pool(name="psH", bufs=4, space="PSUM"))
        psT_pool = ctx.enter_context(tc.tile_pool(name="psT", bufs=1, space="PSUM"))
        ps1_pool = ctx.enter_context(tc.tile_pool(name="ps1", bufs=2, space="PSUM"))
        psA_pool = ctx.enter_context(tc.tile_pool(name="psA", bufs=1, space="PSUM"))

        # ---- resident constants ------------------------------------------------
        eye = const_pool.tile([128, 128], f32, tag="eye")
        nc.sync.dma_start(eye[:], d_eye.ap())
        inT = const_pool.tile([128, KC0 * 128], f32, tag="inT")
        nc.sync.dma_start(inT[:].rearrange("p (k b) -> p k b", k=KC0),
                          d_inT.ap().rearrange("(k p) b -> p k b", p=128))
        bnbig = const_pool.tile([128, 1], f32, tag="bnbig")
        nc.vector.memset(bnbig[:], -BIG)

        # PE warmup: junk transposes (dep only on eye) keep the tensor engine
        # continuously busy from ~1us so the H0 matmuls cost at full p-state.
        warm_ps = psT_pool.tile([128, 128], f32, tag="psT", name="warm")
        for _ in range(N_WARM):
            nc.tensor.transpose(warm_ps[:], eye[:], eye[:])

        # ---- H0 = inputs @ W0 (exact fp32), stored transposed [feat, batch] ----
        H0T = state_pool.tile([128, KC1 * 128], f32, tag="H0T")
        phs = [psH_pool.tile([128, NS], f32, tag="psH", name=f"psH{n}")
               for n in range(F1 // NS)]
        w0ts = []
        for k in range(KC0):
            w0t = w0_pool.tile([128, F1], f32, tag="w0t")
            nc.sync.dma_start(w0t[:], d_W0.ap()[k * 128:(k + 1) * 128, :])
            w0ts.append(w0t)
            if k == 0:
                # sacrificial 1-wide matmuls gated on the first W0 chunk: they
                # soak up the early (unramped) dispatch slots so every real H0
                # matmul below is costed after the PE ramp window.
                for _ in range(4):
                    nc.tensor.matmul(warm_ps[:, 0:1], inT[:, 0:128],
                                     w0t[:, 0:1], start=True, stop=True)
            for n in range(F1 // NS):
                nc.tensor.matmul(phs[n][:],
                                 inT[:, k * 128:(k + 1) * 128],
                                 w0t[:, n * NS:(n + 1) * NS],
                                 start=(k == 0), stop=(k == KC0 - 1))
        W1sb = const_pool.tile([128, KC1 * SH], f32r, tag="W1sb")
        nc.sync.dma_start(W1sb[:].rearrange("p (k s) -> p k s", k=KC1),
                          d_W1s.ap().rearrange("(k p) s -> p k s", p=128))
        W2sb = const_pool.tile([128, (SH // 128) * F3], f32r, tag="W2sb")
        nc.sync.dma_start(W2sb[:].rearrange("p (k o) -> p k o", k=SH // 128),
                          d_W2s.ap().rearrange("(k p) o -> p k o", p=128))

        # transpose H0 -> H0T with two alternating PSUM scratch tiles (psT and
        # the A-accumulator bank, which is first written with start=True at
        # t=2) and the SBUF copies split between DVE and ACT to shorten the
        # serial chain
        A_ps = psA_pool.tile([128, SH], f32, tag="psA")
        tscr = [warm_ps, A_ps]
        for n in range(F1 // NS):
            htmp = htmp_pool.tile([128, NS], f32, tag="htmp")
            nc.scalar.copy(htmp[:], phs[n][:])
            for j in range(NS // 128):
                kk = n * (NS // 128) + j
                tp = tscr[kk % 2]
                nc.tensor.transpose(tp[:, 0:128], htmp[:, j * 128:(j + 1) * 128],
                                    eye[:])
                if kk % 2 == 0:
                    nc.vector.tensor_copy(H0T[:, kk * 128:(kk + 1) * 128],
                                          tp[:, 0:128])
                else:
                    nc.scalar.copy(H0T[:, kk * 128:(kk + 1) * 128], tp[:, 0:128])

        # c_t-scaled identity tiles for the A accumulation + alpha-scaled
        # identity for the S1 decay matmul, built on device (DVE idle here)
        ctis = const_pool.tile([128, T * 128], f32r, tag="ctis")
        coefs = _coeffs()
        for t in range(T):
            nc.vector.tensor_scalar(ctis[:, t * 128:(t + 1) * 128], eye[:],
                                    float(coefs[t]), None, AL.mult)
        alphaI = const_pool.tile([128, 128], f32r, tag="alphaI")
        nc.vector.tensor_scalar(alphaI[:], eye[:], ALPHA, None, AL.mult)

        # ---- state -------------------------------------------------------------
        MP = [state_pool.tile([128, F1], f32, tag=f"MP{i}", name=f"MP{i}")
              for i in range(2)]                      # layer-0 membrane (pre-reset)
        NS0 = [state_pool.tile([128, F1], f32r, tag=f"NS0{i}", name=f"NS0{i}")
               for i in range(2)]                     # spk0 in {0,1}
        S1sb = [state_pool.tile([128, SH], f32r, tag=f"S1sb{i}", name=f"S1sb{i}")
                for i in range(2)]                    # layer-1 synapse (SBUF copy)
        G1 = state_pool.tile([128, SH], f32, tag="G1")  # LIF1 intermediate
        M1P = [state_pool.tile([128, SH], f32, tag=f"M1P{i}", name=f"M1P{i}")
               for i in range(2)]                     # layer-1 membrane (pre-reset)
        NS1 = [state_pool.tile([128, SH], f32r, tag=f"NS1{i}", name=f"NS1{i}")
               for i in range(3)]                     # spk1 ring (A-mm lags 2)

        nc.vector.memset(MP[1][:], 0.0)
        nc.vector.memset(M1P[1][:], 0.0)
        # f32r memset is invalid ISA; zero S1sb via a dtype-converting copy
        nc.scalar.copy(S1sb[1][:], M1P[1][:])

        S1ps = [ps1_pool.tile([128, SH], f32, tag="ps1", name=f"S1ps{i}")
                for i in range(2)]

        def a_mm(t):
            # layer-2 accumulation for step t (emitted 2 steps late in PE order)
            nc.tensor.matmul(A_ps[:], ctis[:, t * 128:(t + 1) * 128],
                             NS1[t % 3][:], start=(t == 0), stop=(t == T - 1),
                             skip_group_check=True)

        # ---- the 50-step recurrence --------------------------------------------
        for t in range(T):
            cu, pr = t % 2, 1 - (t % 2)
            # layer-0 LIF (DVE only), then spike extraction on ACT + Pool
            for s in range(2):
                sl = slice(s * (F1 // 2), (s + 1) * (F1 // 2))
                nc.vector._custom_dve(LIF, out=MP[cu][:, sl], in0=MP[pr][:, sl],
                                      in1=H0T[:, sl], s0=BETA)
            nc.scalar.activation(NS0[cu][:, 0:1024], MP[cu][:, 0:1024],
                                 AF.Sigmoid, bias=bnbig[:], scale=BIG)
            nc.scalar.activation(NS0[cu][:, 1024:F1], MP[cu][:, 1024:F1],
                                 AF.Sigmoid, bias=bnbig[:], scale=BIG)

            # layer-1 matmul group: S1ps = spk0 @ W1s + alpha * S1(t-1)
            for k in range(KC1):
                nc.tensor.matmul(S1ps[cu][:],
                                 NS0[cu][:, k * 128:(k + 1) * 128],
                                 W1sb[:, k * SH:(k + 1) * SH],
                                 start=(k == 0), stop=False)
            nc.tensor.matmul(S1ps[cu][:], alphaI[:], S1sb[pr][:],
                             start=False, stop=True)
            if t >= 2:
                a_mm(t - 2)

            # S1 back to SBUF (ACT) for the next decay matmul; layer-1 LIF as
            # three plain Pool ops (STT is not a legal Pool opcode), spike
            # extraction on Pool
            nc.scalar.copy(S1sb[cu][:], S1ps[cu][:])
            nc.gpsimd.tensor_scalar(G1[:], M1P[pr][:], 1.0, BETA,
                                    AL.is_le, AL.mult)
            nc.gpsimd.tensor_tensor(G1[:], G1[:], M1P[pr][:], AL.mult)
            nc.gpsimd.tensor_tensor(M1P[cu][:], G1[:], S1sb[cu][:], AL.add)
            nc.gpsimd.tensor_scalar(NS1[t % 3][:], M1P[cu][:], 1.0, None, AL.is_gt)

        a_mm(T - 2)
        a_mm(T - 1)

        # ---- final: OUT_partial = A @ W2s ---------------------------------------
        A_sb = out_pool.tile([128, SH], f32, tag="Asb")
        nc.vector.tensor_copy(A_sb[:], A_ps[:])
        AT = out_pool.tile([128, (SH // 128) * 128], f32r, tag="AT")
        for j in range(SH // 128):
            tp = tscr[j % 2]
            nc.tensor.transpose(tp[:, 0:128], A_sb[:, j * 128:(j + 1) * 128],
                                eye[:])
            if j % 2 == 0:
                nc.vector.tensor_copy(AT[:, j * 128:(j + 1) * 128], tp[:, 0:128])
            else:
                nc.scalar.copy(AT[:, j * 128:(j + 1) * 128], tp[:, 0:128])
        pout = psH_pool.tile([128, F3], f32, tag="psH")
        for j in range(SH // 128):
            nc.tensor.matmul(pout[:],
                             AT[:, j * 128:(j + 1) * 128],
                             W2sb[:, j * F3:(j + 1) * F3],
                             start=(j == 0), stop=(j == SH // 128 - 1))
        outsb = out_pool.tile([128, F3], f32, tag="outsb")
        nc.scalar.copy(outsb[:], pout[:])
        nc.sync.dma_start(d_out.ap(), outsb[:])

    nc.compile()
    return nc


_NC_CACHE = []


def _get_nc():
    if not _NC_CACHE:
        _NC_CACHE.append(_build())
    return _NC_CACHE[0]


def kernel(inputs, W0, W1, W2):
    inputs = np.asarray(inputs, dtype=np.float32)
    W0 = np.asarray(W0, dtype=np.float32)
    W1 = np.asarray(W1, dtype=np.float32)
    W2 = np.asarray(W2, dtype=np.float32)

    nc = _get_nc()
    inT = np.ascontiguousarray(inputs.T)
    eye = np.eye(128, dtype=np.float32)
    in_maps = []
    for c in range(N_CORES):
        in_maps.append({
            "inT": inT,
            "W0": W0,
            "W1s": np.ascontiguousarray(W1[:, c * SH:(c + 1) * SH]),
            "W2s": np.ascontiguousarray(W2[c * SH:(c + 1) * SH, :]),
            "EYE": eye,
        })
    try:
        res = bass_utils.run_bass_kernel_spmd(nc, in_maps,
                                              core_ids=list(range(N_CORES)))
    except Exception:
        res = bass_utils.run_bass_kernel_spmd(nc, in_maps,
                                              core_ids=list(range(N_CORES)))
    out = np.zeros((B, F3), dtype=np.float32)
    for c in range(N_CORES):
        out += res.results[c]["OUT"]
    return out


# revision 16
# speedup vs baseline: 1.0317x; 1.0317x over previous
"""Trainium2 Bass kernel: 3-layer spiking NN (DSNN) forward, 50 timesteps.

Strategy (8 NeuronCores, no inter-core communication inside the time loop):
  - Layer-0 drive H0 = inputs @ W0 is constant over time -> computed once on
    each core in exact fp32 (layer-0 spike trains are chaotically sensitive
    to H0 error), transposed to [feature, batch] layout via PE transposes.
    The PE is warmed up with junk transposes + sacrificial micro-matmuls so
    the H0 fp32 matmuls are costed at the ramped p-state.
  - Per-step work is spread over all four compute engines:
      DVE : layer-0 LIF only (fused custom op, 2 slabs of 1024)
      ACT : spike extraction for features [0:1792) + S1 PSUM->SBUF copy
      Pool: spike extraction [1792:2048) (is_gt), layer-1 LIF as two
            scalar_tensor_tensor primitives, spk1 extraction (is_gt)
      PE  : 16 accumulating fp32r matmuls (spk0 @ W1 shard) plus an
            alpha-scaled identity matmul that folds the synaptic decay
            S1' = alpha*S1 + h1 directly into the PSUM accumulation, and
            the layer-2 A-accumulation matmul (delayed 2 steps in PE
            program order so the cross-engine chain never stalls the PE).
  - Layer 2 is linear: m2(T) = sum_t c_t * (spk1(t) @ W2).  A = sum c_t*spk1
    accumulates in a pinned PSUM bank; one A @ W2[shard, :] matmul at the end.
  - Host sums the 8 partial [128, 512] outputs.
"""

import numpy as np
from contextlib import ExitStack

import concourse.bacc as bacc
import concourse.bass as bass
import concourse.mybir as mybir
import concourse.tile as tile
from concourse import bass_utils
from concourse import dve_ops as _DOPS
from concourse.dve_spec import Spec, Src0, Src1, C0, One, lower as _dve_lower
from concourse.dve_uop import DveOpSpec as _DveOpSpec

ALPHA = 0.9
BETA = 0.85
T = 50
B = 128            # batch
F0, F1, F3 = 1024, 2048, 512
N_CORES = 8
SH = F1 // N_CORES  # 256 layer-1 features per core
KC0 = F0 // 128     # 8 contraction chunks for H0
KC1 = F1 // 128     # 16 contraction chunks for layer-1 matmul
NS = 512            # H0 free-dim slice width
BIG = 1.0e30
SPLIT_B = 1536      # ACT handles spk0 cols [0:SPLIT_B), Pool [SPLIT_B:F1)

f32 = mybir.dt.float32
f32r = mybir.dt.float32r
AL = mybir.AluOpType
AF = mybir.ActivationFunctionType

N_WARM = 16         # PE warmup transposes before the H0 fp32 matmuls


def _register_lif():
    """Fused LIF step on the pre-reset membrane:
    out = beta * (in0 * (in0 <= 1)) + in1  (reset-gate, leak, drive)."""
    name = "DSNN_LIF_STEP"
    for op in _DOPS.OPS:
        if op.name == name:
            return op
    body = (Src0 * (Src0 <= One)) * C0 + Src1
    spec = Spec(body=body,
                reference=lambda in0, in1, s0, s1, imm2:
                    ((in0 * (in0 <= 1.0)) * np.float32(s0) + in1).astype(np.float32))
    row = max(_DOPS._SUB_OPCODE_FOR_NAME.values()) + 1
    _DOPS._SUB_OPCODE_FOR_NAME[name] = row
    shas = {}
    for ver in ("v3", "v4"):
        uops = _dve_lower(spec, ver=ver)
        shas[ver] = _DveOpSpec(name=name, opcode=row, uops=uops, rd1_en=True).sha(ver)
    op = _DOPS.DveOp(name, spec, subdim=False, uops_sha=shas)
    _DOPS.OPS.append(op)
    _DOPS.CUSTOM_DVE_SPECS[name] = spec
    return op


LIF = _register_lif()


def _coeffs():
    # m2(T) = sum_{t=1..T} c[t-1] * h2(t)
    c = np.zeros(T, dtype=np.float64)
    for s in range(T):
        tau = s + 1
        c[s] = sum(BETA ** (T - t) * ALPHA ** (t - tau) for t in range(tau, T + 1))
    return c.astype(np.float32)


def _build():
    nc = bacc.Bacc("TRN2", target_bir_lowering=False, debug=False)
    d_inT = nc.dram_tensor("inT", [F0, B], f32, kind="ExternalInput")
    d_W0 = nc.dram_tensor("W0", [F0, F1], f32, kind="ExternalInput")
    d_W1s = nc.dram_tensor("W1s", [F1, SH], f32r, kind="ExternalInput")
    d_W2s = nc.dram_tensor("W2s", [SH, F3], f32r, kind="ExternalInput")
    d_eye = nc.dram_tensor("EYE", [128, 128], f32, kind="ExternalInput")
    d_out = nc.dram_tensor("OUT", [B, F3], f32, kind="ExternalOutput")

    with tile.TileContext(nc) as tc, ExitStack() as ctx:
        const_pool = ctx.enter_context(tc.tile_pool(name="const", bufs=1))
        state_pool = ctx.enter_context(tc.tile_pool(name="state", bufs=1))
        w0_pool = ctx.enter_context(tc.tile_pool(name="w0s", bufs=3))
        htmp_pool = ctx.enter_context(tc.tile_pool(name="htmp", bufs=2))
        out_pool = ctx.enter_context(tc.tile_pool(name="outp", bufs=1))
        psH_pool = ctx.enter_context(tc.tile_pool(name="psH", bufs=4, space="PSUM"))
        psT_pool = ctx.enter_context(tc.tile_pool(name="psT", bufs=1, space="PSUM"))
        ps1_pool = ctx.enter_context(tc.tile_pool(name="ps1", bufs=2, space="PSUM"))
        psA_pool = ctx.enter_context(tc.tile_pool(name="psA", bufs=1, space="PSUM"))

        # ---- resident constants ------------------------------------------------
        eye = const_pool.tile([128, 128], f32, tag="eye")
        nc.sync.dma_start(eye[:], d_eye.ap())
        inT = const_pool.tile([128, KC0 * 128], f32, tag="inT")
        nc.sync.dma_start(inT[:].rearrange("p (k b) -> p k b", k=KC0),
                          d_inT.ap().rearrange("(k p) b -> p k b", p=128))
        bnbig = const_pool.tile([128, 1], f32, tag="bnbig")
        nc.vector.memset(bnbig[:], -BIG)

        # PE warmup: junk transposes (dep only on eye) keep the tensor engine
        # continuously busy from ~1us so the H0 matmuls cost at full p-state.
        warm_ps = psT_pool.tile([128, 128], f32, tag="psT", name="warm")
        for _ in range(N_WARM):
            nc.tensor.transpose(warm_ps[:], eye[:], eye[:])

        # ---- H0 = inputs @ W0 (exact fp32), stored transposed [feat, batch] ----
        H0T = state_pool.tile([128, KC1 * 128], f32, tag="H0T")
        phs = [psH_pool.tile([128, NS], f32, tag="psH", name=f"psH{n}")
               for n in range(F1 // NS)]
        w0ts = []
        for k in range(KC0):
            w0t = w0_pool.tile([128, F1], f32, tag="w0t")
            nc.sync.dma_start(w0t[:], d_W0.ap()[k * 128:(k + 1) * 128, :])
            w0ts.append(w0t)
            if k == 0:
                # sacrificial 1-wide matmuls gated on the first W0 chunk: they
                # soak up the early (unramped) dispatch slots so every real H0
                # matmul below is costed after the PE ramp window.
                for _ in range(4):
                    nc.tensor.matmul(warm_ps[:, 0:1], inT[:, 0:128],
                                     w0t[:, 0:1], start=True, stop=True)
            for n in range(F1 // NS):
                nc.tensor.matmul(phs[n][:],
                                 inT[:, k * 128:(k + 1) * 128],
                                 w0t[:, n * NS:(n + 1) * NS],
                                 start=(k == 0), stop=(k == KC0 - 1))
        W1sb = const_pool.tile([128, KC1 * SH], f32r, tag="W1sb")
        nc.sync.dma_start(W1sb[:].rearrange("p (k s) -> p k s", k=KC1),
                          d_W1s.ap().rearrange("(k p) s -> p k s", p=128))
        W2sb = const_pool.tile([128, (SH // 128) * F3], f32r, tag="W2sb")
        nc.sync.dma_start(W2sb[:].rearrange("p (k o) -> p k o", k=SH // 128),
                          d_W2s.ap().rearrange("(k p) o -> p k o", p=128))

        # transpose H0 -> H0T with two alternating PSUM scratch tiles (psT and
        # the A-accumulator bank, which is first written with start=True at
        # t=2) and the SBUF copies split between DVE and ACT to shorten the
        # serial chain
        A_ps = psA_pool.tile([128, SH], f32, tag="psA")
        tscr = [warm_ps, A_ps]
        for n in range(F1 // NS):
            htmp = htmp_pool.tile([128, NS], f32, tag="htmp")
            nc.scalar.copy(htmp[:], phs[n][:])
            for j in range(NS // 128):
                kk = n * (NS // 128) + j
                tp = tscr[kk % 2]
                nc.tensor.transpose(tp[:, 0:128], htmp[:, j * 128:(j + 1) * 128],
                                    eye[:])
                if kk % 2 == 0:
                    nc.vector.tensor_copy(H0T[:, kk * 128:(kk + 1) * 128],
                                          tp[:, 0:128])
                else:
                    nc.scalar.copy(H0T[:, kk * 128:(kk + 1) * 128], tp[:, 0:128])

        # c_t-scaled identity tiles for the A accumulation + alpha-scaled
        # identity for the S1 decay matmul, built on device (DVE idle here)
        ctis = const_pool.tile([128, T * 128], f32r, tag="ctis")
        coefs = _coeffs()
        for t in range(T):
            nc.vector.tensor_scalar(ctis[:, t * 128:(t + 1) * 128], eye[:],
                                    float(coefs[t]), None, AL.mult)
        alphaI = const_pool.tile([128, 128], f32r, tag="alphaI")
        nc.vector.tensor_scalar(alphaI[:], eye[:], ALPHA, None, AL.mult)

        # ---- state -------------------------------------------------------------
        MP = [state_pool.tile([128, F1], f32, tag=f"MP{i}", name=f"MP{i}")
              for i in range(2)]                      # layer-0 membrane (pre-reset)
        NS0 = [state_pool.tile([128, F1], f32r, tag=f"NS0{i}", name=f"NS0{i}")
               for i in range(2)]                     # spk0 in {0,1}
        S1sb = [state_pool.tile([128, SH], f32r, tag=f"S1sb{i}", name=f"S1sb{i}")
                for i in range(2)]                    # layer-1 synapse (SBUF copy)
        M1P = [state_pool.tile([128, SH], f32, tag=f"M1P{i}", name=f"M1P{i}")
               for i in range(2)]                     # layer-1 membrane (pre-reset)
        NS1 = [state_pool.tile([128, SH], f32r, tag=f"NS1{i}", name=f"NS1{i}")
               for i in range(3)]                     # spk1 ring (A-mm lags 2)

        nc.vector.memset(MP[1][:], 0.0)
        nc.vector.memset(M1P[1][:], 0.0)
        # f32r memset is invalid ISA; zero S1sb via a dtype-converting copy
        nc.scalar.copy(S1sb[1][:], M1P[1][:])

        S1ps = [ps1_pool.tile([128, SH], f32, tag="ps1", name=f"S1ps{i}")
                for i in range(2)]

        def a_mm(t):
            # layer-2 accumulation for step t (emitted 2 steps late in PE order)
            nc.tensor.matmul(A_ps[:], ctis[:, t * 128:(t + 1) * 128],
                             NS1[t % 3][:], start=(t == 0), stop=(t == T - 1),
                             skip_group_check=True)

        # ---- the 50-step recurrence --------------------------------------------
        for t in range(T):
            cu, pr = t % 2, 1 - (t % 2)
            # layer-0 LIF (DVE only), then spike extraction on ACT + Pool
            for s in range(2):
                sl = slice(s * (F1 // 2), (s + 1) * (F1 // 2))
                nc.vector._custom_dve(LIF, out=MP[cu][:, sl], in0=MP[pr][:, sl],
                                      in1=H0T[:, sl], s0=BETA)
            nc.scalar.activation(NS0[cu][:, 0:1024], MP[cu][:, 0:1024],
                                 AF.Sigmoid, bias=bnbig[:], scale=BIG)
            nc.scalar.activation(NS0[cu][:, 1024:SPLIT_B], MP[cu][:, 1024:SPLIT_B],
                                 AF.Sigmoid, bias=bnbig[:], scale=BIG)
            nc.gpsimd.tensor_scalar(NS0[cu][:, SPLIT_B:F1], MP[cu][:, SPLIT_B:F1],
                                    1.0, None, AL.is_gt)

            # layer-1 matmul group: S1ps = spk0 @ W1s + alpha * S1(t-1)
            for k in range(KC1):
                nc.tensor.matmul(S1ps[cu][:],
                                 NS0[cu][:, k * 128:(k + 1) * 128],
                                 W1sb[:, k * SH:(k + 1) * SH],
                                 start=(k == 0), stop=False)
            nc.tensor.matmul(S1ps[cu][:], alphaI[:], S1sb[pr][:],
                             start=False, stop=True)
            if t >= 2:
                a_mm(t - 2)

            # S1 back to SBUF (ACT) for the next decay matmul; layer-1 LIF on
            # DVE straight from PSUM (STT is not a legal Pool opcode); spike
            # extraction on Pool
            nc.scalar.copy(S1sb[cu][:], S1ps[cu][:])
            nc.vector._custom_dve(LIF, out=M1P[cu][:], in0=M1P[pr][:],
                                  in1=S1ps[cu][:], s0=BETA)
            nc.gpsimd.tensor_scalar(NS1[t % 3][:], M1P[cu][:], 1.0, None, AL.is_gt)

        a_mm(T - 2)
        a_mm(T - 1)

        # ---- final: OUT_partial = A @ W2s ---------------------------------------
        A_sb = out_pool.tile([128, SH], f32, tag="Asb")
        nc.vector.tensor_copy(A_sb[:], A_ps[:])
        AT = out_pool.tile([128, (SH // 128) * 128], f32r, tag="AT")
        for j in range(SH // 128):
            tp = tscr[j % 2]
            nc.tensor.transpose(tp[:, 0:128], A_sb[:, j * 128:(j + 1) * 128],
                                eye[:])
            if j % 2 == 0:
                nc.vector.tensor_copy(AT[:, j * 128:(j + 1) * 128], tp[:, 0:128])
            else:
                nc.scalar.copy(AT[:, j * 128:(j + 1) * 128], tp[:, 0:128])
        pout = psH_pool.tile([128, F3], f32, tag="psH")
        for j in range(SH // 128):
            nc.tensor.matmul(pout[:],
                             AT[:, j * 128:(j + 1) * 128],
                             W2sb[:, j * F3:(j + 1) * F3],
                             start=(j == 0), stop=(j == SH // 128 - 1))
        outsb = out_pool.tile([128, F3], f32, tag="outsb")
        nc.scalar.copy(outsb[:], pout[:])
        nc.sync.dma_start(d_out.ap(), outsb[:])

    nc.compile()
    return nc


_NC_CACHE = []


def _get_nc():
    if not _NC_CACHE:
        _NC_CACHE.append(_build())
    return _NC_CACHE[0]


def kernel(inputs, W0, W1, W2):
    inputs = np.asarray(inputs, dtype=np.float32)
    W0 = np.asarray(W0, dtype=np.float32)
    W1 = np.asarray(W1, dtype=np.float32)
    W2 = np.asarray(W2, dtype=np.float32)

    nc = _get_nc()
    inT = np.ascontiguousarray(inputs.T)
    eye = np.eye(128, dtype=np.float32)
    in_maps = []
    for c in range(N_CORES):
        in_maps.append({
            "inT": inT,
            "W0": W0,
            "W1s": np.ascontiguousarray(W1[:, c * SH:(c + 1) * SH]),
            "W2s": np.ascontiguousarray(W2[c * SH:(c + 1) * SH, :]),
            "EYE": eye,
        })
    try:
        res = bass_utils.run_bass_kernel_spmd(nc, in_maps,
                                              core_ids=list(range(N_CORES)))
    except Exception:
        res = bass_utils.run_bass_kernel_spmd(nc, in_maps,
                                              core_ids=list(range(N_CORES)))
    out = np.zeros((B, F3), dtype=np.float32)
    for c in range(N_CORES):
        out += res.results[c]["OUT"]
    return out


# revision 20
# speedup vs baseline: 1.0670x; 1.0342x over previous
"""Trainium2 Bass kernel: 3-layer spiking NN (DSNN) forward, 50 timesteps.

Strategy (8 NeuronCores, no inter-core communication inside the time loop):
  - Layer-0 drive H0 = inputs @ W0 is constant over time -> computed once on
    each core in exact fp32 (layer-0 spike trains are chaotically sensitive
    to H0 error), transposed to [feature, batch] layout via PE transposes.
    The PE is warmed up with junk transposes + sacrificial micro-matmuls so
    the H0 fp32 matmuls are costed at the ramped p-state.
  - Per-step work is spread over all four compute engines:
      DVE : layer-0 LIF (fused custom op, 2 slabs of 1024) + layer-1 LIF
            (same fused op, reading S1 straight from PSUM)
      ACT : spike extraction for features [0:1536) + S1 PSUM->SBUF copy
      Pool: spike extraction [1536:2048) (is_gt), spk1 extraction (is_gt)
      PE  : 16 accumulating fp32r matmuls (spk0 @ W1 shard) plus an
            alpha-scaled identity matmul that folds the synaptic decay
            S1' = alpha*S1 + h1 directly into the PSUM accumulation, and
            the layer-2 A-accumulation matmul (delayed 2 steps in PE
            program order so the cross-engine chain never stalls the PE).
  - Layer 2 is linear: m2(T) = sum_t c_t * (spk1(t) @ W2).  A = sum c_t*spk1
    accumulates in a pinned PSUM bank; one A @ W2[shard, :] matmul at the end.
  - Host sums the 8 partial [128, 512] outputs.
"""

import numpy as np
from contextlib import ExitStack

import concourse.bacc as bacc
import concourse.bass as bass
import concourse.mybir as mybir
import concourse.tile as tile
from concourse import bass_utils
from concourse import dve_ops as _DOPS
from concourse.dve_spec import Spec, Src0, Src1, C0, One, lower as _dve_lower
from concourse.dve_uop import DveOpSpec as _DveOpSpec

ALPHA = 0.9
BETA = 0.85
T = 50
B = 128            # batch
F0, F1, F3 = 1024, 2048, 512
N_CORES = 8
SH = F1 // N_CORES  # 256 layer-1 features per core
KC0 = F0 // 128     # 8 contraction chunks for H0
KC1 = F1 // 128     # 16 contraction chunks for layer-1 matmul
NS = 512            # H0 free-dim slice width
BIG = 1.0e30
SPLIT_B = 1536      # ACT handles spk0 cols [0:SPLIT_B), Pool [SPLIT_B:F1)

f32 = mybir.dt.float32
f32r = mybir.dt.float32r
AL = mybir.AluOpType
AF = mybir.ActivationFunctionType

N_WARM = 16         # PE warmup transposes before the H0 fp32 matmuls


def _register_lif():
    """Fused LIF step on the pre-reset membrane:
    out = beta * (in0 * (in0 <= 1)) + in1  (reset-gate, leak, drive)."""
    name = "DSNN_LIF_STEP"
    for op in _DOPS.OPS:
        if op.name == name:
            return op
    body = (Src0 * (Src0 <= One)) * C0 + Src1
    spec = Spec(body=body,
                reference=lambda in0, in1, s0, s1, imm2:
                    ((in0 * (in0 <= 1.0)) * np.float32(s0) + in1).astype(np.float32))
    row = max(_DOPS._SUB_OPCODE_FOR_NAME.values()) + 1
    _DOPS._SUB_OPCODE_FOR_NAME[name] = row
    shas = {}
    for ver in ("v3", "v4"):
        uops = _dve_lower(spec, ver=ver)
        shas[ver] = _DveOpSpec(name=name, opcode=row, uops=uops, rd1_en=True).sha(ver)
    op = _DOPS.DveOp(name, spec, subdim=False, uops_sha=shas)
    _DOPS.OPS.append(op)
    _DOPS.CUSTOM_DVE_SPECS[name] = spec
    return op


LIF = _register_lif()


def _coeffs():
    # m2(T) = sum_{t=1..T} c[t-1] * h2(t)
    c = np.zeros(T, dtype=np.float64)
    for s in range(T):
        tau = s + 1
        c[s] = sum(BETA ** (T - t) * ALPHA ** (t - tau) for t in range(tau, T + 1))
    return c.astype(np.float32)


def _build():
    nc = bacc.Bacc("TRN2", target_bir_lowering=False, debug=False)
    d_inT = nc.dram_tensor("inT", [F0, B], f32, kind="ExternalInput")
    d_W0 = nc.dram_tensor("W0", [F0, F1], f32, kind="ExternalInput")
    d_W1s = nc.dram_tensor("W1s", [F1, SH], f32r, kind="ExternalInput")
    d_W2s = nc.dram_tensor("W2s", [SH, F3], f32r, kind="ExternalInput")
    d_eye = nc.dram_tensor("EYE", [128, 128], f32, kind="ExternalInput")
    d_out = nc.dram_tensor("OUT", [B, F3], f32, kind="ExternalOutput")

    with tile.TileContext(nc) as tc, ExitStack() as ctx:
        const_pool = ctx.enter_context(tc.tile_pool(name="const", bufs=1))
        state_pool = ctx.enter_context(tc.tile_pool(name="state", bufs=1))
        w0_pool = ctx.enter_context(tc.tile_pool(name="w0s", bufs=3))
        htmp_pool = ctx.enter_context(tc.tile_pool(name="htmp", bufs=2))
        out_pool = ctx.enter_context(tc.tile_pool(name="outp", bufs=1))
        psH_pool = ctx.enter_context(tc.tile_pool(name="psH", bufs=4, space="PSUM"))
        psT_pool = ctx.enter_context(tc.tile_pool(name="psT", bufs=1, space="PSUM"))
        ps1_pool = ctx.enter_context(tc.tile_pool(name="ps1", bufs=2, space="PSUM"))
        psA_pool = ctx.enter_context(tc.tile_pool(name="psA", bufs=1, space="PSUM"))

        # ---- resident constants ------------------------------------------------
        eye = const_pool.tile([128, 128], f32, tag="eye")
        nc.sync.dma_start(eye[:], d_eye.ap())
        inT = const_pool.tile([128, KC0 * 128], f32, tag="inT")
        nc.sync.dma_start(inT[:].rearrange("p (k b) -> p k b", k=KC0),
                          d_inT.ap().rearrange("(k p) b -> p k b", p=128))
        bnbig = const_pool.tile([128, 1], f32, tag="bnbig")
        nc.vector.memset(bnbig[:], -BIG)

        # PE warmup: junk transposes (dep only on eye) keep the tensor engine
        # continuously busy from ~1us so the H0 matmuls cost at full p-state.
        warm_ps = psT_pool.tile([128, 128], f32, tag="psT", name="warm")
        for _ in range(N_WARM):
            nc.tensor.transpose(warm_ps[:], eye[:], eye[:])

        # ---- H0 = inputs @ W0 (exact fp32), stored transposed [feat, batch] ----
        H0T = state_pool.tile([128, KC1 * 128], f32, tag="H0T")
        phs = [psH_pool.tile([128, NS], f32, tag="psH", name=f"psH{n}")
               for n in range(F1 // NS)]
        w0ts = []
        for k in range(KC0):
            w0t = w0_pool.tile([128, F1], f32, tag="w0t")
            nc.sync.dma_start(w0t[:], d_W0.ap()[k * 128:(k + 1) * 128, :])
            w0ts.append(w0t)
            if k == 0:
                # sacrificial 1-wide matmuls gated on the first W0 chunk: they
                # soak up the early (unramped) dispatch slots so every real H0
                # matmul below is costed after the PE ramp window.
                for _ in range(4):
                    nc.tensor.matmul(warm_ps[:, 0:1], inT[:, 0:128],
                                     w0t[:, 0:1], start=True, stop=True)
            for n in range(F1 // NS):
                nc.tensor.matmul(phs[n][:],
                                 inT[:, k * 128:(k + 1) * 128],
                                 w0t[:, n * NS:(n + 1) * NS],
                                 start=(k == 0), stop=(k == KC0 - 1))
        W1sb = const_pool.tile([128, KC1 * SH], f32r, tag="W1sb")
        nc.sync.dma_start(W1sb[:].rearrange("p (k s) -> p k s", k=KC1),
                          d_W1s.ap().rearrange("(k p) s -> p k s", p=128))
        W2sb = const_pool.tile([128, (SH // 128) * F3], f32r, tag="W2sb")
        nc.sync.dma_start(W2sb[:].rearrange("p (k o) -> p k o", k=SH // 128),
                          d_W2s.ap().rearrange("(k p) o -> p k o", p=128))

        # transpose H0 -> H0T with two alternating PSUM scratch tiles (psT and
        # the A-accumulator bank, which is first written with start=True at
        # t=2) and the SBUF copies split between DVE and ACT to shorten the
        # serial chain
        A_ps = psA_pool.tile([128, SH], f32, tag="psA")
        tscr = [warm_ps, A_ps]
        for n in range(F1 // NS):
            htmp = htmp_pool.tile([128, NS], f32, tag="htmp")
            nc.scalar.copy(htmp[:], phs[n][:])
            for j in range(NS // 128):
                kk = n * (NS // 128) + j
                tp = tscr[kk % 2]
                nc.tensor.transpose(tp[:, 0:128], htmp[:, j * 128:(j + 1) * 128],
                                    eye[:])
                if kk % 2 == 0:
                    nc.vector.tensor_copy(H0T[:, kk * 128:(kk + 1) * 128],
                                          tp[:, 0:128])
                else:
                    nc.scalar.copy(H0T[:, kk * 128:(kk + 1) * 128], tp[:, 0:128])

        # c_t-scaled identity tiles for the A accumulation + alpha-scaled
        # identity for the S1 decay matmul, built on device (DVE idle here)
        ctis = const_pool.tile([128, T * 128], f32r, tag="ctis")
        coefs = _coeffs()
        for t in range(T):
            nc.vector.tensor_scalar(ctis[:, t * 128:(t + 1) * 128], eye[:],
                                    float(coefs[t]), None, AL.mult)
        # scaled identities for the expanded layer-1 recurrence
        #   M(t) = (a+b)M(t-1) - b R(t-1) - ab M(t-2) + ab R(t-2) + h1(t)
        # (S1 substituted out; R = M * spk1 is the reset correction)
        idc = []
        for c in (ALPHA + BETA, -BETA, -ALPHA * BETA, ALPHA * BETA):
            ti = const_pool.tile([128, 128], f32r, tag=f"idc{len(idc)}")
            nc.vector.tensor_scalar(ti[:], eye[:], float(c), None, AL.mult)
            idc.append(ti)

        # ---- state -------------------------------------------------------------
        MP = [state_pool.tile([128, F1], f32, tag=f"MP{i}", name=f"MP{i}")
              for i in range(2)]                      # layer-0 membrane (pre-reset)
        NS0 = [state_pool.tile([128, F1], f32r, tag=f"NS0{i}", name=f"NS0{i}")
               for i in range(2)]                     # spk0 in {0,1}
        M1sb = [state_pool.tile([128, SH], f32r, tag=f"M1sb{i}", name=f"M1sb{i}")
                for i in range(3)]                    # layer-1 membrane ring
        Rsb = [state_pool.tile([128, SH], f32r, tag=f"Rsb{i}", name=f"Rsb{i}")
               for i in range(3)]                     # reset correction M*spk1
        NS1 = [state_pool.tile([128, SH], f32r, tag=f"NS1{i}", name=f"NS1{i}")
               for i in range(3)]                     # spk1 ring (A-mm lags 2)
        zf = state_pool.tile([128, SH], f32, tag="zf")

        nc.vector.memset(MP[1][:], 0.0)
        nc.vector.memset(zf[:], 0.0)
        # f32r memset is invalid ISA; zero the rings via dtype-converting copies
        for ti in (M1sb[1], M1sb[2], Rsb[1], Rsb[2]):
            nc.scalar.copy(ti[:], zf[:])

        S1ps = [ps1_pool.tile([128, SH], f32, tag="ps1", name=f"S1ps{i}")
                for i in range(2)]

        def a_mm(t):
            # layer-2 accumulation for step t (emitted 2 steps late in PE order)
            nc.tensor.matmul(A_ps[:], ctis[:, t * 128:(t + 1) * 128],
                             NS1[t % 3][:], start=(t == 0), stop=(t == T - 1),
                             skip_group_check=True)

        # ---- the 50-step recurrence --------------------------------------------
        for t in range(T):
            cu, pr = t % 2, 1 - (t % 2)
            # layer-0 LIF (DVE only), then spike extraction on ACT + Pool
            for s in range(2):
                sl = slice(s * (F1 // 2), (s + 1) * (F1 // 2))
                nc.vector._custom_dve(LIF, out=MP[cu][:, sl], in0=MP[pr][:, sl],
                                      in1=H0T[:, sl], s0=BETA)
            nc.scalar.activation(NS0[cu][:, 0:1024], MP[cu][:, 0:1024],
                                 AF.Sigmoid, bias=bnbig[:], scale=BIG)
            nc.scalar.activation(NS0[cu][:, 1024:SPLIT_B], MP[cu][:, 1024:SPLIT_B],
                                 AF.Sigmoid, bias=bnbig[:], scale=BIG)
            nc.gpsimd.tensor_scalar(NS0[cu][:, SPLIT_B:F1], MP[cu][:, SPLIT_B:F1],
                                    1.0, None, AL.is_gt)

            # layer-1 matmul group: M1ps = spk0 @ W1s + the 4 recurrence terms
            for k in range(KC1):
                nc.tensor.matmul(S1ps[cu][:],
                                 NS0[cu][:, k * 128:(k + 1) * 128],
                                 W1sb[:, k * SH:(k + 1) * SH],
                                 start=(k == 0), stop=False)
            for i, src in enumerate((M1sb[(t - 1) % 3], Rsb[(t - 1) % 3],
                                     M1sb[(t - 2) % 3], Rsb[(t - 2) % 3])):
                nc.tensor.matmul(S1ps[cu][:], idc[i][:], src[:],
                                 start=False, stop=(i == 3))
            if t >= 2:
                a_mm(t - 2)

            # membrane to SBUF (ACT); spike + reset correction on Pool
            nc.scalar.copy(M1sb[t % 3][:], S1ps[cu][:])
            nc.gpsimd.tensor_scalar(NS1[t % 3][:], M1sb[t % 3][:], 1.0, None,
                                    AL.is_gt)
            nc.gpsimd.tensor_tensor(Rsb[t % 3][:], M1sb[t % 3][:],
                                    NS1[t % 3][:], AL.mult)

        a_mm(T - 2)
        a_mm(T - 1)

        # ---- final: OUT_partial = A @ W2s ---------------------------------------
        A_sb = out_pool.tile([128, SH], f32, tag="Asb")
        nc.vector.tensor_copy(A_sb[:], A_ps[:])
        AT = out_pool.tile([128, (SH // 128) * 128], f32r, tag="AT")
        for j in range(SH // 128):
            tp = tscr[j % 2]
            nc.tensor.transpose(tp[:, 0:128], A_sb[:, j * 128:(j + 1) * 128],
                                eye[:])
            if j % 2 == 0:
                nc.vector.tensor_copy(AT[:, j * 128:(j + 1) * 128], tp[:, 0:128])
            else:
                nc.scalar.copy(AT[:, j * 128:(j + 1) * 128], tp[:, 0:128])
        pout = psH_pool.tile([128, F3], f32, tag="psH")
        for j in range(SH // 128):
            nc.tensor.matmul(pout[:],
                             AT[:, j * 128:(j + 1) * 128],
                             W2sb[:, j * F3:(j + 1) * F3],
                             start=(j == 0), stop=(j == SH // 128 - 1))
        outsb = out_pool.tile([128, F3], f32, tag="outsb")
        nc.scalar.copy(outsb[:], pout[:])
        nc.sync.dma_start(d_out.ap(), outsb[:])

    nc.compile()
    return nc


_NC_CACHE = []


def _get_nc():
    if not _NC_CACHE:
        _NC_CACHE.append(_build())
    return _NC_CACHE[0]


def kernel(inputs, W0, W1, W2):
    inputs = np.asarray(inputs, dtype=np.float32)
    W0 = np.asarray(W0, dtype=np.float32)
    W1 = np.asarray(W1, dtype=np.float32)
    W2 = np.asarray(W2, dtype=np.float32)

    nc = _get_nc()
    inT = np.ascontiguousarray(inputs.T)
    eye = np.eye(128, dtype=np.float32)
    in_maps = []
    for c in range(N_CORES):
        in_maps.append({
            "inT": inT,
            "W0": W0,
            "W1s": np.ascontiguousarray(W1[:, c * SH:(c + 1) * SH]),
            "W2s": np.ascontiguousarray(W2[c * SH:(c + 1) * SH, :]),
            "EYE": eye,
        })
    try:
        res = bass_utils.run_bass_kernel_spmd(nc, in_maps,
                                              core_ids=list(range(N_CORES)))
    except Exception:
        res = bass_utils.run_bass_kernel_spmd(nc, in_maps,
                                              core_ids=list(range(N_CORES)))
    out = np.zeros((B, F3), dtype=np.float32)
    for c in range(N_CORES):
        out += res.results[c]["OUT"]
    return out
